# revision 1
# baseline (speedup 1.0000x reference)
"""Trainium2 kernel for nn_ConvNN_2D_Spatial_K_N_Location.

Strategy (8 NeuronCores):
  - The two KNN-conv layers (irregular top-9 selection/gather, ~6% of FLOPs)
    run on host in fp32 with reference-exact tie-breaking.
  - The dominant FC stack runs on the 8 cores with the fc1 contraction dim
    (32768) sharded 8 ways in f16: core i gets h2.T[F_i] and fw1.T[F_i]
    (8 MB each), computes fp32 partials for all 1024 batch rows, an
    on-device ReduceScatter leaves core i with final fc1 batch rows
    [128i:128(i+1)], then fused bias+relu and fc2 produce its 128x10 slice.
    H2D traffic is 128 MB f16 total vs 1.15 GB for a replicated-fw1 plan
    (the host<->device link is the bottleneck at ~25-70 MB/s).
"""
import numpy as np

import concourse.bass as bass
import concourse.tile as tile
from concourse import bacc, mybir
from concourse.bass_utils import run_bass_kernel_spmd

try:
    import os as _os
    import jax as _jax
    _os.makedirs("/tmp/jax_cc_cache", exist_ok=True)
    _jax.config.update("jax_compilation_cache_dir", "/tmp/jax_cc_cache")
    _jax.config.update("jax_persistent_cache_min_compile_time_secs", 0)
except Exception:
    pass

K, N, SCALE = 9, 8, 2
F16 = np.dtype(np.float16)
NCORES = 8
B = 1024
F = 32768
FSH = F // NCORES      # 4096
U = 1024
O2 = 10

_CACHE = {}


# ---------------------------------------------------------------- host conv
def _unshuffle(x, s):
    B_, C, H, W = x.shape
    return x.reshape(B_, C, H//s, s, W//s, s).transpose(0, 1, 3, 5, 2, 4).reshape(B_, C*s*s, H//s, W//s)


def _shuffle(x, s):
    B_, C, H, W = x.shape
    return x.reshape(B_, C//(s*s), s, s, H, W).transpose(0, 1, 4, 2, 5, 3).reshape(B_, C//(s*s), H*s, W*s)


def _conv_nn(x, w, b):
    x = _unshuffle(x, SCALE)
    B_, C, H, W = x.shape
    gy, gx = np.meshgrid(np.linspace(0., 1., H, dtype=np.float32),
                         np.linspace(0., 1., W, dtype=np.float32), indexing='ij')
    loc = np.broadcast_to(np.stack([gy, gx])[None], (B_, 2, H, W)).astype(np.float32)
    x = np.concatenate([x, loc], 1)
    Cf = C + 2
    xf = x.reshape(B_, Cf, H*W)
    ih = np.linspace(0, H-1, N).astype(np.int32)
    iw = np.linspace(0, W-1, N).astype(np.int32)
    samp = x[:, :, ih][:, :, :, iw].reshape(B_, Cf, N*N)
    # ranking key: d2 minus the per-token norm (constant in n, preserves order)
    s2 = np.einsum('bcn,bcn->bn', samp, samp)
    xfT = np.ascontiguousarray(xf.transpose(0, 2, 1))
    score = s2[:, None, :] - 2.0 * np.matmul(xfT, samp)
    # top-K nearest, ties broken toward lower candidate index (== jax top_k)
    part = np.argpartition(score, K, axis=2)[:, :, :K]
    pv = np.take_along_axis(score, part, axis=2)
    o9 = np.lexsort((part, pv), axis=2)
    idx = np.take_along_axis(part, o9, axis=2)
    sampT = np.ascontiguousarray(samp.transpose(0, 2, 1))
    w_kcT = np.ascontiguousarray(
        np.ascontiguousarray(w.transpose(0, 2, 1)).reshape(w.shape[0], K * Cf).T)
    O = w.shape[0]
    T = H * W
    out = np.empty((B_, O, T), np.float32)
    # block over batch so the gather table stays cache-resident
    bb = 64
    barange = np.arange(bb)[:, None, None]
    for s in range(0, B_, bb):
        ngb = sampT[s:s + bb][barange, idx[s:s + bb], :]    # (bb, T, K, Cf)
        r = ngb.reshape(bb * T, K * Cf) @ w_kcT
        r += b
        out[s:s + bb] = r.reshape(bb, T, O).transpose(0, 2, 1)
    return _shuffle(out.reshape(B_, O, H, W), SCALE)


# ---------------------------------------------------------------- device fc
def _build_fc_kernel():
    if 'nc' in _CACHE:
        return _CACHE['nc']
    nc = bacc.Bacc("TRN2", target_bir_lowering=False, debug=False,
                   enable_asserts=False, num_devices=NCORES)
    f32 = mybir.dt.float32
    f16 = mybir.dt.float16
    h2ti = nc.dram_tensor("h2ti", (FSH, B), f16, kind="ExternalInput").ap()
    fw1s = nc.dram_tensor("fw1s", (FSH, U), f16, kind="ExternalInput").ap()
    fb1t = nc.dram_tensor("fb1t", (128, 8), f32, kind="ExternalInput").ap()
    fw2t = nc.dram_tensor("fw2t", (U, O2), f16, kind="ExternalInput").ap()
    fb2r = nc.dram_tensor("fb2r", (1, O2), f16, kind="ExternalInput").ap()
    onesr = nc.dram_tensor("onesr", (1, 128), f16, kind="ExternalInput").ap()
    ident = nc.dram_tensor("ident", (128, 128), f32, kind="ExternalInput").ap()
    outt = nc.dram_tensor("outt", (128, O2), f32, kind="ExternalOutput").ap()

    NCH = FSH // 128       # 32 feature chunks per core

    with tile.TileContext(nc) as tc:
        with tc.tile_pool(name="wres", bufs=1) as wres, \
             tc.tile_pool(name="small", bufs=1) as spool, \
             tc.tile_pool(name="stage", bufs=2) as stpool, \
             tc.tile_pool(name="acts", bufs=1) as apool, \
             tc.tile_pool(name="ps", bufs=2, space="PSUM") as pspool, \
             tc.tile_pool(name="pst", bufs=2, space="PSUM") as ptpool, \
             tc.tile_pool(name="dram", bufs=1, space="DRAM") as dram:

            # resident weights + activations: 64KB + 64KB per partition
            wtile = wres.tile([128, NCH * U], f16)
            htile = wres.tile([128, NCH * B], f16)
            for c in range(NCH):
                nc.sync.dma_start(wtile[:, bass.ts(c, U)], fw1s[bass.ts(c, 128), :])
                nc.sync.dma_start(htile[:, bass.ts(c, B)], h2ti[bass.ts(c, 128), :])

            ones_t = spool.tile([1, 128], f16)
            nc.sync.dma_start(ones_t[:], onesr[:, :])
            fb1_t = spool.tile([128, 8], f32)
            nc.sync.dma_start(fb1_t[:], fb1t[:, :])
            fb2_t = spool.tile([1, O2], f16)
            nc.sync.dma_start(fb2_t[:], fb2r[:, :])
            id_t = spool.tile([128, 128], f32)
            nc.sync.dma_start(id_t[:], ident[:, :])
            fw2_t = spool.tile([128, 8 * O2], f16)
            for c in range(8):
                nc.sync.dma_start(fw2_t[:, bass.ts(c, O2)], fw2t[bass.ts(c, 128), :])

            bounce_in = dram.tile([B, U], f32)
            bounce_out = dram.tile([128, U], f32)

            # fc1 partials over all 8 batch blocks
            for j in range(NCORES):
                psum = pspool.tile([128, U], f32)
                for c in range(NCH):
                    lhsT = htile[:, c * B + j * 128: c * B + (j + 1) * 128]
                    for half in range(2):
                        nc.tensor.matmul(psum[:, bass.ts(half, 512)],
                                         lhsT=lhsT,
                                         rhs=wtile[:, c * U + half * 512: c * U + (half + 1) * 512],
                                         start=(c == 0), stop=(c == NCH - 1))
                stg = stpool.tile([128, U], f32)
                nc.scalar.copy(stg[:], psum[:])
                nc.sync.dma_start(bounce_in[j * 128:(j + 1) * 128, :], stg[:])

            nc.gpsimd.collective_compute(
                "ReduceScatter", mybir.AluOpType.add,
                replica_groups=[list(range(NCORES))],
                ins=[bounce_in.opt()], outs=[bounce_out.opt()],
            )

            h1raw = apool.tile([128, U], f32)
            nc.sync.dma_start(h1raw[:], bounce_out[:])

            # transpose 128x128 blocks; relu(x + fb1) fused on the way out
            h1T = apool.tile([128, U], f16)
            for c in range(8):
                pt = ptpool.tile([128, 128], f32)
                nc.tensor.transpose(pt[:], h1raw[:, bass.ts(c, 128)], id_t[:])
                nc.scalar.activation(h1T[:, bass.ts(c, 128)], pt[:],
                                     mybir.ActivationFunctionType.Relu,
                                     bias=fb1_t[:, c:c + 1])

            psum2 = ptpool.tile([128, O2], f32)
            for c in range(8):
                nc.tensor.matmul(psum2[:], lhsT=h1T[:, bass.ts(c, 128)],
                                 rhs=fw2_t[:, bass.ts(c, O2)],
                                 start=(c == 0), stop=False)
            nc.tensor.matmul(psum2[:], lhsT=ones_t[:], rhs=fb2_t[:],
                             start=False, stop=True)

            out_t = apool.tile([128, O2], f32)
            nc.scalar.copy(out_t[:], psum2[:])
            nc.sync.dma_start(outt[:, :], out_t[:])

    nc.compile()
    _CACHE['nc'] = nc
    return nc


def kernel(x, w1, b1, w2, b2, fw1, fb1, fw2, fb2):
    import time as _time
    import sys as _sys
    _t0 = _time.time()

    def _mark(label):
        print(f"[kernel] {label}: {_time.time() - _t0:.2f}s", file=_sys.stderr, flush=True)

    x = np.asarray(x, np.float32)
    # host: the two KNN-conv layers (exact fp32 ranking, reference tie-break)
    h1 = np.maximum(_conv_nn(x, np.asarray(w1, np.float32), np.asarray(b1, np.float32)), 0)
    _mark("conv1")
    h2 = np.maximum(_conv_nn(h1, np.asarray(w2, np.float32), np.asarray(b2, np.float32)), 0)
    _mark("conv2")
    h2 = h2.reshape(B, -1)                              # (1024, 32768)

    nc = _build_fc_kernel()
    _mark("bass ready")
    fw1 = np.asarray(fw1, np.float32)
    fb1t = np.ascontiguousarray(np.asarray(fb1, np.float32).reshape(8, 128).T)
    fw2t = np.asarray(fw2, np.float32).T.astype(F16)
    fb2r = np.asarray(fb2, np.float32).reshape(1, O2).astype(F16)
    onesr = np.ones((1, 128), F16)
    ident = np.eye(128, dtype=np.float32)
    in_maps = []
    for i in range(NCORES):
        sl = slice(i * FSH, (i + 1) * FSH)
        in_maps.append(dict(h2ti=h2[:, sl].T.astype(F16),
                            fw1s=fw1[:, sl].T.astype(F16),
                            fb1t=fb1t, fw2t=fw2t, fb2r=fb2r,
                            onesr=onesr, ident=ident))
    _mark("prep in_maps")
    res = run_bass_kernel_spmd(nc, in_maps, core_ids=list(range(NCORES)))
    _mark("spmd run")
    out = np.empty((B, O2), np.float32)
    for i in range(NCORES):
        out[i * 128:(i + 1) * 128] = res.results[i]["outt"]
    return out



# revision 19
# speedup vs baseline: 1.2984x; 1.2984x over previous
"""Trainium2 kernel for nn_ConvNN_2D_Spatial_K_N_Location — full device version.

Strategy (8 NeuronCores, batch-sharded conv + feature-sharded fc1):
  - Each core runs both KNN-conv layers for its 128 batches entirely on
    device. Top-9 selection uses the DVE max8/match_replace chain; the
    rank of every candidate is recovered by counting threshold compares
    (broadcast-AP tensor op + innermost-axis reduce); the rank-dependent
    Conv1d aggregation is evaluated through 9 "moment masks" sel*(r-c)^p
    (split Lagrange basis on ranks 0-4 / 5-8, exact small ints in f16)
    so the gather becomes 9 dense matmuls per batch.
  - Pixel shuffle/unshuffle between the layers cancels; the final
    shuffle+flatten is folded into a host-side permutation of fw1.
  - fc1 is contraction-sharded: AllToAll redistributes conv output
    (batch-shard -> feature-shard), each core computes a 1024x1024
    partial, ReduceScatter returns final batch rows, then bias+relu+fc2.
"""
import os
import numpy as np

import concourse.bass as bass
import concourse.tile as tile
from concourse import bacc, mybir
from concourse.bass_utils import run_bass_kernel_spmd
from concourse.bass_types import AP

try:
    import jax as _jax
    os.makedirs("/tmp/jax_cc_cache", exist_ok=True)
    _jax.config.update("jax_compilation_cache_dir", "/tmp/jax_cc_cache")
    _jax.config.update("jax_persistent_cache_min_compile_time_secs", 0)
except Exception:
    pass

F16 = np.dtype(np.float16)
NCORES = 8
B = 1024
NB = B // NCORES          # 128 batches per core
NPAIR = NB // 2           # 64
NGRP1 = NB // 4           # 32 groups of 4 (layer 1, 32-row blocks)
T = 256                   # tokens per image (16x16)
NC = 64                   # candidates
U = 1024
O2out = 10

_CACHE = {}
_LAST_RES = None


# ------------------------------------------------------------------ helpers
def fap(sl, pattern):
    """Keep the partition pair of an AP slice, replace free pattern."""
    return AP(tensor=sl.tensor, offset=sl.offset, ap=[sl.ap[0]] + pattern)


def _cand_copy_pieces(ngrp, src_grp_stride, dst_grp_stride):
    """4 strided pieces gathering candidate columns (8x8 grid of
    [0,2,..,12,15]^2 positions) out of each group's 256 token columns."""
    # (a-count, b-count, src_off, dst_off, src_pat, dst_pat)
    return [
        (7, 7, 0, 0,
         [[src_grp_stride, ngrp], [32, 7], [2, 7]],
         [[dst_grp_stride, ngrp], [8, 7], [1, 7]]),
        (7, 1, 15, 7,
         [[src_grp_stride, ngrp], [32, 7]],
         [[dst_grp_stride, ngrp], [8, 7]]),
        (1, 7, 240, 56,
         [[src_grp_stride, ngrp], [2, 7]],
         [[dst_grp_stride, ngrp], [1, 7]]),
        (1, 1, 255, 63,
         [[src_grp_stride, ngrp], [1, 1]],
         [[dst_grp_stride, ngrp], [1, 1]]),
    ]


# ------------------------------------------------------------------ builder
def _build_nc():
    if "nc" in _CACHE:
        return _CACHE["nc"]
    nc = bacc.Bacc("TRN2", target_bir_lowering=False, debug=False,
                   enable_asserts=False, num_devices=NCORES)
    f32 = mybir.dt.float32
    f16 = mybir.dt.float16
    AO = mybir.AluOpType

    # per-core inputs
    x1s = nc.dram_tensor("x1s", (128, NGRP1 * 256), f32, kind="ExternalInput").ap()
    fw1s = nc.dram_tensor("fw1s", (4096, U), f16, kind="ExternalInput").ap()
    # shared inputs
    wb1 = nc.dram_tensor("wb1", (32, 9 * 64), f32, kind="ExternalInput").ap()
    wb2 = nc.dram_tensor("wb2", (128, 9 * 128), f32, kind="ExternalInput").ap()
    lws1 = nc.dram_tensor("lws1", (64, 9 * 64), f32, kind="ExternalInput").ap()
    lws2 = nc.dram_tensor("lws2", (64, 9 * 128), f32, kind="ExternalInput").ap()
    d2tm = nc.dram_tensor("d2tm", (128, 128), f32, kind="ExternalInput").ap()
    idt32 = nc.dram_tensor("idt32", (128, 128), f32, kind="ExternalInput").ap()
    bc1 = nc.dram_tensor("bc1", (128, 4), f32, kind="ExternalInput").ap()
    bc2 = nc.dram_tensor("bc2", (128, 2), f32, kind="ExternalInput").ap()
    b1c = nc.dram_tensor("b1c", (64, 1), f32, kind="ExternalInput").ap()
    b2c = nc.dram_tensor("b2c", (128, 1), f32, kind="ExternalInput").ap()
    ones32 = nc.dram_tensor("ones32", (1, 128), f32, kind="ExternalInput").ap()
    ones16 = nc.dram_tensor("ones16", (1, 128), f16, kind="ExternalInput").ap()
    fb1t = nc.dram_tensor("fb1t", (128, 8), f32, kind="ExternalInput").ap()
    fw2t = nc.dram_tensor("fw2t", (128, 80), f16, kind="ExternalInput").ap()
    fb2r = nc.dram_tensor("fb2r", (1, O2out), f16, kind="ExternalInput").ap()
    outt = nc.dram_tensor("outt", (NB, O2out), f32, kind="ExternalOutput").ap()
    DEBUG = bool(os.environ.get("BASSK_DEBUG"))
    if DEBUG:
        g1dump = nc.dram_tensor("g1dump", (128, NPAIR * 256), f32, kind="ExternalOutput").ap()
        g2dump = nc.dram_tensor("g2dump", (128, NB * 256), f16, kind="ExternalOutput").ap()

    with tile.TileContext(nc) as tc:
        with tc.tile_pool(name="dram", bufs=1, space="DRAM") as dram:
            g2d = dram.tile([128, NB * T], f16)          # conv2 out (o2, b*t)
            g1d = dram.tile([128, NPAIR * T], mybir.dt.float32)  # conv1 out f32
            a2out = dram.tile([128, NB * T], f16)        # alltoall result
            rs_in = dram.tile([B, U], f32)
            rs_out = dram.tile([NB, U], f32)

            # ======================================================== conv
            with tc.tile_pool(name="consts", bufs=1) as cst, \
                 tc.tile_pool(name="xp", bufs=1) as xp, \
                 tc.tile_pool(name="g1p", bufs=1) as g1p, \
                 tc.tile_pool(name="g2p", bufs=1) as g2p, \
                 tc.tile_pool(name="wk", bufs=2) as wk, \
                 tc.tile_pool(name="tcmp", bufs=8) as tcmp, \
                 tc.tile_pool(name="psc", bufs=2, space="PSUM") as psc_p, \
                 tc.tile_pool(name="ptp", bufs=2, space="PSUM") as ptp_p, \
                 tc.tile_pool(name="pws", bufs=1, space="PSUM") as pws_p, \
                 tc.tile_pool(name="pagg", bufs=2, space="PSUM") as pagg_p:

                def ctile(nm, shape, dt_, src):
                    t_ = cst.tile(shape, dt_, name=nm, tag=nm)
                    nc.sync.dma_start(t_[:], src[:, :])
                    return t_

                idt32t = ctile("idt32t", [128, 128], f32, idt32)
                d2tmt = ctile("d2tmt", [128, 128], f32, d2tm)
                wb1t = ctile("wb1t", [32, 576], f32, wb1)
                wb2t = ctile("wb2t", [128, 1152], f32, wb2)
                lws1t = ctile("lws1t", [64, 576], f32, lws1)
                lws2t = ctile("lws2t", [64, 1152], f32, lws2)
                bc1t = ctile("bc1t", [128, 4], f32, bc1)
                bc2t = ctile("bc2t", [128, 2], f32, bc2)
                b1ct = ctile("b1ct", [64, 1], f32, b1c)
                b2ct = ctile("b2ct", [128, 1], f32, b2c)
                ones32t = ctile("ones32t", [1, 128], f32, ones32)
                ones16t = ctile("ones16t", [1, 128], f16, ones16)


                # ---------------- generic conv layer ----------------
                def conv_layer(layer):
                    if layer == 1:
                        nblk, bstr, ngrp, O = 4, 32, NGRP1, 64   # blockdiag count, block stride
                        xdram, wbt, lwst, bct, bcol = x1s, wb1t, lws1t, b1ct, bc1t
                    else:
                        nblk, bstr, ngrp, O = 2, 64, NPAIR, 128
                        xdram, wbt, lwst, bct, bcol = g1d, wb2t, lws2t, b2ct, bc2t
                    P = 128
                    NPG = nblk // 2                               # pairs per group
                    WSW = 9 * O                                   # ws width

                    for grp in range(ngrp):
                        gw = nblk * 64
                        # stream this group's tokens from DRAM
                        xg = wk.tile([128, 256], f32, tag="xg", bufs=3)
                        nc.sync.dma_start(xg[:], xdram[:, grp * 256:(grp + 1) * 256])
                        # candidate gather for this group (unscaled f32)
                        ss = wk.tile([128, 64], f32, tag="sscur", bufs=3)
                        for (na, nbp, so, do, sp, dp) in _cand_copy_pieces(1, 256, 64):
                            nc.vector.tensor_copy(
                                fap(ss[0:P, do:do + 1], dp[1:]),
                                fap(xg[0:P, so:so + 1], sp[1:]))
                        # block-diagonal lhs (2x scale folded here)
                        bd = wk.tile([P, gw], f32, tag="bd")
                        nc.vector.memset(bd[:], 0.0)
                        for g in range(nblk):
                            nc.vector.tensor_scalar(
                                bd[bstr * g:bstr * (g + 1), 64 * g:64 * (g + 1)],
                                ss[bstr * g:bstr * (g + 1), 0:64],
                                2.0, None, op0=AO.mult)
                        # squares + -s2 row (1, gw)
                        sq = wk.tile([P, 64], f32, tag="sq")
                        nc.vector.tensor_tensor(sq[:], ss[:, 0:64], ss[:, 0:64], op=AO.mult)
                        ps2t = ptp_p.tile([nblk, 64], f32, tag="ptp")
                        nc.tensor.matmul(ps2t[:], lhsT=bcol[:], rhs=sq[:],
                                         start=True, stop=True)
                        s2sb = wk.tile([nblk, 64], f32, tag="s2sb")
                        nc.scalar.copy(s2sb[:], ps2t[:])
                        s2fl = wk.tile([1, gw], f32, tag="s2fl")
                        nc.sync.dma_start(
                            fap(s2fl[0:1, 0:1], [[64, nblk], [1, 64]]), s2sb[:])

                        tcms = [tcmp.tile([128, 256], f32, tag="tcm", name=f"tcm{layer}_{grp}_{i}")
                                for i in range(NPG)]
                        for half in range(2):
                            psc = psc_p.tile([128, gw], f32, tag="psc")
                            nc.tensor.matmul(
                                psc[:], lhsT=xg[:, half * 128:half * 128 + 128],
                                rhs=bd[:], start=True, stop=False)
                            nc.tensor.matmul(psc[:], lhsT=ones32t[:, 0:128],
                                             rhs=s2fl[:], start=False, stop=True)
                            negsc = wk.tile([128, gw], f32, tag="negsc")
                            nc.vector.tensor_tensor(
                                fap(negsc[0:128, 0:1], [[64, nblk], [1, 64]]),
                                fap(psc[0:128, 0:1], [[64, nblk], [1, 64]]),
                                fap(d2tmt[0:128, half * 64:half * 64 + 1], [[0, nblk], [1, 64]]),
                                op=AO.subtract)
                            vbuf = wk.tile([128, nblk * 16], f32, tag="vbuf")
                            mrt = wk.tile([128, gw], f32, tag="mrt")
                            for g in range(nblk):
                                nc.vector.max(out=vbuf[:, g * 16:g * 16 + 8],
                                              in_=negsc[:, g * 64:(g + 1) * 64])
                                nc.vector.match_replace(
                                    out=mrt[:, g * 64:(g + 1) * 64],
                                    in_to_replace=vbuf[:, g * 16:g * 16 + 8],
                                    in_values=negsc[:, g * 64:(g + 1) * 64],
                                    imm_value=-1e30)
                                nc.vector.max(out=vbuf[:, g * 16 + 8:g * 16 + 16],
                                              in_=mrt[:, g * 64:(g + 1) * 64])
                            for pq in range(NPG):
                                Ct = wk.tile([128, 1152], f16, tag="ct")
                                nc.vector.tensor_tensor(
                                    fap(Ct[0:128, 0:1], [[576, 2], [9, 64], [1, 9]]),
                                    fap(negsc[0:128, pq * 128:pq * 128 + 1], [[64, 2], [1, 64], [0, 9]]),
                                    fap(vbuf[0:128, pq * 32:pq * 32 + 1], [[16, 2], [0, 64], [1, 9]]),
                                    op=AO.is_lt)
                                Tt = wk.tile([128, 128], f32, tag="tt")
                                nc.vector.tensor_reduce(
                                    Tt[:], fap(Ct[0:128, 0:1], [[9, 128], [1, 9]]),
                                    axis=mybir.AxisListType.X, op=AO.add)
                                ptp = ptp_p.tile([128, 128], f32, tag="ptp")
                                nc.tensor.transpose(ptp[:], Tt[:], idt32t[:])
                                nc.scalar.copy(tcms[pq][:, half * 128:half * 128 + 128], ptp[:])

                        for pq in range(NPG):
                            pairg = grp * NPG + pq
                            Tcm = tcms[pq]
                            M = wk.tile([128, 9 * 256], f32, tag="m")
                            selt = wk.tile([128, 256], f32, tag="selt")
                            tca = wk.tile([128, 256], f32, tag="tca")
                            tcb = wk.tile([128, 256], f32, tag="tcb")
                            nc.vector.tensor_scalar(M[:, 0:256], Tcm[:], 4.5, None, op0=AO.is_le)
                            nc.vector.tensor_scalar(selt[:], Tcm[:], 8.5, None, op0=AO.is_le)
                            nc.vector.tensor_tensor(M[:, 5 * 256:6 * 256], selt[:], M[:, 0:256], op=AO.subtract)
                            nc.vector.tensor_scalar(tca[:], Tcm[:], -2.0, None, op0=AO.add)
                            nc.vector.tensor_scalar(tcb[:], Tcm[:], -6.0, None, op0=AO.add)
                            for p in range(1, 5):
                                nc.vector.tensor_tensor(M[:, p * 256:(p + 1) * 256],
                                                        M[:, (p - 1) * 256:p * 256], tca[:], op=AO.mult)
                            for p in range(6, 9):
                                nc.vector.tensor_tensor(M[:, p * 256:(p + 1) * 256],
                                                        M[:, (p - 1) * 256:p * 256], tcb[:], op=AO.mult)
                            ws = wk.tile([128, WSW], f32, tag="ws")
                            for gg in range(2):
                                g = pq * 2 + gg
                                b = grp * nblk + g
                                if layer == 1:
                                    sswk = wk.tile([32, 64], f32, tag="sswk")
                                    nc.vector.tensor_copy(
                                        sswk[:], ss[bstr * g:bstr * (g + 1), 0:64])
                                    lhs_ws = sswk[:]
                                    rhs_ws = wbt
                                else:
                                    lhs_ws = ss[bstr * g:bstr * (g + 1), 0:64]
                                    rhs_ws = wbt[bstr * g:bstr * (g + 1), :]
                                for piece in range((WSW + 511) // 512):
                                    lo = piece * 512
                                    hi = min(lo + 512, WSW)
                                    pws = pws_p.tile([64, 512], f32, tag="pws")
                                    nc.tensor.matmul(pws[:, 0:hi - lo], lhsT=lhs_ws,
                                                     rhs=rhs_ws[:, lo:hi],
                                                     start=True, stop=True)
                                    nc.vector.tensor_tensor(
                                        ws[64 * gg:64 * gg + 64, lo:hi],
                                        pws[:, 0:hi - lo], lwst[:, lo:hi], op=AO.add)
                                pagg = pagg_p.tile([O, 256], f32, tag="pagg")
                                for p in range(9):
                                    nc.tensor.matmul(pagg[:], lhsT=ws[64 * gg:64 * gg + 64,
                                                                      p * O:(p + 1) * O],
                                                     rhs=M[64 * gg:64 * gg + 64, p * 256:(p + 1) * 256],
                                                     start=(p == 0), stop=(p == 8))
                                if layer == 1:
                                    if gg == 0:
                                        g1stg = wk.tile([128, 256], mybir.dt.float32,
                                                        tag="g1stg", name=f"g1stg_{grp}_{pq}")
                                    nc.scalar.activation(
                                        g1stg[64 * gg:64 * gg + 64, :], pagg[:],
                                        mybir.ActivationFunctionType.Relu, bias=bct[:, 0:1])
                                    if gg == 1:
                                        nc.sync.dma_start(
                                            g1d[:, pairg * 256:(pairg + 1) * 256], g1stg[:])
                                else:
                                    g2stg = wk.tile([128, 256], mybir.dt.float16, tag="g2stg")
                                    nc.scalar.activation(g2stg[:], pagg[:],
                                                         mybir.ActivationFunctionType.Relu,
                                                         bias=bct[:, 0:1])
                                    nc.sync.dma_start(g2d[:, b * 256:(b + 1) * 256], g2stg[:])

                conv_layer(1)
                conv_layer(2)
                if DEBUG:
                    nc.sync.dma_start(g1dump[:, :], g1d[:, :])
                    nc.sync.dma_start(g2dump[:, :], g2d[:, :])

            nc.gpsimd.collective_compute(
                "AllToAll", mybir.AluOpType.bypass,
                replica_groups=[list(range(NCORES))],
                ins=[g2d.opt()], outs=[a2out.opt()],
            )

            # ======================================================== fc
            f32 = mybir.dt.float32
            f16 = mybir.dt.float16
            with tc.tile_pool(name="fcw", bufs=1) as fcw, \
                 tc.tile_pool(name="fcs", bufs=2) as fcs, \
                 tc.tile_pool(name="cst2", bufs=1) as cst2, \
                 tc.tile_pool(name="pfc", bufs=2, space="PSUM") as pfc_p, \
                 tc.tile_pool(name="ptp2", bufs=2, space="PSUM") as ptp2_p:

                idt32b = cst2.tile([128, 128], f32)
                nc.sync.dma_start(idt32b[:], idt32[:, :])
                fb1tb = cst2.tile([128, 8], f32)
                nc.sync.dma_start(fb1tb[:], fb1t[:, :])
                fw2tb = cst2.tile([128, 80], f16)
                nc.sync.dma_start(fw2tb[:], fw2t[:, :])
                fb2rb = cst2.tile([1, O2out], f16)
                nc.sync.dma_start(fb2rb[:], fb2r[:, :])
                ones16b = cst2.tile([1, 128], f16)
                nc.sync.dma_start(ones16b[:], ones16[:, :])

                fw1sb = fcw.tile([128, 32 * U], f16)
                nc.sync.dma_start(
                    fw1sb[:],
                    fap(fw1s[0:128, 0:1], [[U * 128, 32], [1, U]]))
                h2sb = fcw.tile([128, 32 * U], f16)
                # restack alltoall output: chunk c = (o2r=c//2, t-half c%2);
                # a2out[16j+o2r, b*256+t]; chunk partitions = t-half, free (j, b)
                for c in range(32):
                    for j in range(8):
                        sl = AP(tensor=a2out.tensor,
                                offset=a2out[16 * j + c // 2:16 * j + c // 2 + 1,
                                             (c % 2) * 128:(c % 2) * 128 + 1].offset,
                                ap=[[1, 128], [256, 128]])
                        nc.sync.dma_start(h2sb[:, c * U + j * 128:c * U + (j + 1) * 128], sl)

                for bt in range(8):
                    for uh in range(2):
                        pfc = pfc_p.tile([128, 512], f32, tag="pfc")
                        for c in range(32):
                            nc.tensor.matmul(
                                pfc[:], lhsT=h2sb[:, c * U + bt * 128:c * U + bt * 128 + 128],
                                rhs=fw1sb[:, c * U + uh * 512:c * U + uh * 512 + 512],
                                start=(c == 0), stop=(c == 31))
                        stg = fcs.tile([128, 512], f32, tag="stg")
                        nc.scalar.copy(stg[:], pfc[:])
                        nc.sync.dma_start(
                            rs_in[bt * 128:(bt + 1) * 128, uh * 512:(uh + 1) * 512], stg[:])

                nc.gpsimd.collective_compute(
                    "ReduceScatter", mybir.AluOpType.add,
                    replica_groups=[list(range(NCORES))],
                    ins=[rs_in.opt()], outs=[rs_out.opt()],
                )

                h1raw = fcs.tile([128, U], f32, tag="h1raw")
                nc.sync.dma_start(h1raw[:], rs_out[:, :])
                h1T = fcs.tile([128, U], f16, tag="h1T")
                for c in range(8):
                    ptp2 = ptp2_p.tile([128, 128], f32, tag="ptp2")
                    nc.tensor.transpose(ptp2[:], h1raw[:, c * 128:(c + 1) * 128], idt32b[:])
                    nc.scalar.activation(h1T[:, c * 128:(c + 1) * 128], ptp2[:],
                                         mybir.ActivationFunctionType.Relu,
                                         bias=fb1tb[:, c:c + 1])
                psum2 = ptp2_p.tile([128, O2out], f32, tag="psum2b")
                for c in range(8):
                    nc.tensor.matmul(psum2[:], lhsT=h1T[:, c * 128:(c + 1) * 128],
                                     rhs=fw2tb[:, c * O2out:(c + 1) * O2out],
                                     start=(c == 0), stop=False)
                nc.tensor.matmul(psum2[:], lhsT=ones16b[:], rhs=fb2rb[:],
                                 start=False, stop=True)
                out_t = fcs.tile([128, O2out], f32, tag="outf")
                nc.scalar.copy(out_t[:], psum2[:])
                nc.sync.dma_start(outt[:, :], out_t[:])

    nc.compile()
    _CACHE["nc"] = nc
    return nc


# ------------------------------------------------------------------ host prep
def _host_shared(w1, b1, w2, b2, fb1, fw2, fb2):
    pos = np.linspace(0., 1., 16).astype(np.float32)
    tt = np.arange(T)
    ly, lx = pos[tt // 16], pos[tt % 16]
    IH = np.linspace(0, 15, 8).astype(np.int32)
    cand_t = (IH[:, None] * 16 + IH[None, :]).reshape(-1)
    cy, cx = ly[cand_t], lx[cand_t]
    d2loc = (ly[:, None] - cy[None, :]) ** 2 + (lx[:, None] - cx[None, :]) ** 2
    d2tm = np.empty((128, 128), np.float32)
    for half in range(2):
        d2tm[:, half * 64:(half + 1) * 64] = d2loc[half * 128:(half + 1) * 128, :]

    VA = np.array([[(r - 2) ** p for p in range(5)] for r in range(5)], np.float64)
    CA = np.linalg.inv(VA)
    VB = np.array([[(r - 6) ** p for p in range(4)] for r in range(5, 9)], np.float64)
    CB = np.linalg.inv(VB)

    def basis(w):  # w (O, Cf, 9) -> Wb (9, O, Cf)
        O, Cf, _ = w.shape
        Wb = np.zeros((9, O, Cf), np.float64)
        for k in range(9):
            if k <= 4:
                for p in range(5):
                    Wb[p] += CA[p, k] * w[:, :, k]
            else:
                for p in range(4):
                    Wb[5 + p] += CB[p, k - 5] * w[:, :, k]
        return Wb

    Wb1 = basis(np.asarray(w1, np.float64))     # (9, 64, 14)
    Wb2 = basis(np.asarray(w2, np.float64))     # (9, 128, 66)

    # feature part, halved (samples are 2x-scaled), replicated per block
    wb1r = np.zeros((32, 576), np.float32)
    for p in range(9):
        wb1r[:12, p * 64:(p + 1) * 64] = Wb1[p, :, :12].T
    wb2r = np.zeros((128, 1152), np.float32)
    for g in range(2):
        for p in range(9):
            wb2r[g * 64:(g + 1) * 64, p * 128:(p + 1) * 128] = \
                Wb2[p, :, :64].T
    # location part: lws[n, p*O+o] = sum_l locval[l,n] * Wb[p,o,Cfeat+l]
    locv = np.stack([cy, cx])                    # (2, 64)
    lws1 = np.zeros((64, 576), np.float32)
    lws2 = np.zeros((64, 1152), np.float32)
    for p in range(9):
        lws1[:, p * 64:(p + 1) * 64] = locv.T @ Wb1[p, :, 12:].T
        lws2[:, p * 128:(p + 1) * 128] = locv.T @ Wb2[p, :, 64:].T

    bc1 = np.zeros((128, 4), np.float32)
    for g in range(4):
        bc1[g * 32:g * 32 + 12, g] = -1.0
    bc2 = np.zeros((128, 2), np.float32)
    for g in range(2):
        bc2[g * 64:(g + 1) * 64, g] = -1.0

    fw2 = np.asarray(fw2, np.float32)
    fw2t = fw2.T.reshape(8, 128, O2out).transpose(1, 0, 2).reshape(128, 80)
    return dict(
        wb1=wb1r, wb2=wb2r,
        lws1=lws1, lws2=lws2, d2tm=d2tm,
        idt32=np.eye(128, dtype=np.float32),
        bc1=bc1, bc2=bc2,
        b1c=np.asarray(b1, np.float32).reshape(64, 1),
        b2c=np.asarray(b2, np.float32).reshape(128, 1),
        ones32=np.ones((1, 128), np.float32),
        ones16=np.ones((1, 128), F16),
        fb1t=np.ascontiguousarray(np.asarray(fb1, np.float32).reshape(8, 128).T),
        fw2t=fw2t.astype(F16),
        fb2r=np.asarray(fb2, np.float32).reshape(1, O2out).astype(F16),
    )


def _phi():
    O2v, HH, WW = np.meshgrid(np.arange(128), np.arange(16), np.arange(16),
                              indexing="ij")
    C2 = O2v // 4
    I = (O2v % 4) // 2
    J = O2v % 2
    return (C2 * 1024 + (2 * HH + I) * 32 + (2 * WW + J)).reshape(-1)


def kernel(x, w1, b1, w2, b2, fw1, fb1, fw2, fb2):
    import time as _time
    import sys as _sys
    _t0 = _time.time()

    def _mark(label):
        print(f"[kernel] {label}: {_time.time() - _t0:.2f}s", file=_sys.stderr, flush=True)

    nc = _build_nc()
    _mark("bass ready")

    x = np.asarray(x, np.float32)
    xu = x.reshape(B, 3, 16, 2, 16, 2).transpose(0, 1, 3, 5, 2, 4).reshape(B, 12, 256)
    shared = _host_shared(w1, b1, w2, b2, fb1, fw2, fb2)
    phi = _phi()
    fw1p = np.asarray(fw1, np.float32).T[phi].astype(F16)   # (32768, 1024)
    _mark("host prep")

    in_maps = []
    for i in range(NCORES):
        xc = xu[i * NB:(i + 1) * NB]
        a = np.zeros((4, 32, NGRP1, 256), np.float32)
        a[:, :12] = xc.reshape(NGRP1, 4, 12, 256).transpose(1, 2, 0, 3)
        x1stack = np.ascontiguousarray(a.reshape(128, NGRP1 * 256))
        m = dict(shared)
        m["x1s"] = x1stack
        m["fw1s"] = fw1p[i * 4096:(i + 1) * 4096]
        in_maps.append(m)
    _mark("in_maps")

    res = run_bass_kernel_spmd(nc, in_maps, core_ids=list(range(NCORES)))
    global _LAST_RES
    _LAST_RES = res
    _mark("spmd run")
    out = np.empty((B, O2out), np.float32)
    for i in range(NCORES):
        out[i * NB:(i + 1) * NB] = res.results[i]["outt"]
    return out


# revision 20
# speedup vs baseline: 4.6227x; 3.5602x over previous
"""Trainium2 kernel for nn_ConvNN_2D_Spatial_K_N_Location — full device version.

Strategy (8 NeuronCores, batch-sharded conv + feature-sharded fc1):
  - Each core runs both KNN-conv layers for its 128 batches entirely on
    device. Top-9 selection uses the DVE max8/match_replace chain; the
    rank of every candidate is recovered by counting threshold compares
    (broadcast-AP tensor op + innermost-axis reduce); the rank-dependent
    Conv1d aggregation is evaluated through 9 "moment masks" sel*(r-c)^p
    (split Lagrange basis on ranks 0-4 / 5-8, exact small ints in f16)
    so the gather becomes 9 dense matmuls per batch.
  - Pixel shuffle/unshuffle between the layers cancels; the final
    shuffle+flatten is folded into a host-side permutation of fw1.
  - fc1 is contraction-sharded: AllToAll redistributes conv output
    (batch-shard -> feature-shard), each core computes a 1024x1024
    partial, ReduceScatter returns final batch rows, then bias+relu+fc2.
"""
import os
import numpy as np

import concourse.bass as bass
import concourse.tile as tile
from concourse import bacc, mybir
from concourse.bass_utils import run_bass_kernel_spmd
from concourse.bass_types import AP

try:
    import jax as _jax
    os.makedirs("/tmp/jax_cc_cache", exist_ok=True)
    _jax.config.update("jax_compilation_cache_dir", "/tmp/jax_cc_cache")
    _jax.config.update("jax_persistent_cache_min_compile_time_secs", 0)
except Exception:
    pass

F16 = np.dtype(np.float16)
NCORES = 8
B = 1024
NB = B // NCORES          # 128 batches per core
NPAIR = NB // 2           # 64
NGRP1 = NB // 4           # 32 groups of 4 (layer 1, 32-row blocks)
T = 256                   # tokens per image (16x16)
NC = 64                   # candidates
U = 1024
O2out = 10

_CACHE = {}
_LAST_RES = None


# ------------------------------------------------------------------ helpers
def fap(sl, pattern):
    """Keep the partition pair of an AP slice, replace free pattern."""
    return AP(tensor=sl.tensor, offset=sl.offset, ap=[sl.ap[0]] + pattern)


def _cand_copy_pieces(ngrp, src_grp_stride, dst_grp_stride):
    """4 strided pieces gathering candidate columns (8x8 grid of
    [0,2,..,12,15]^2 positions) out of each group's 256 token columns."""
    # (a-count, b-count, src_off, dst_off, src_pat, dst_pat)
    return [
        (7, 7, 0, 0,
         [[src_grp_stride, ngrp], [32, 7], [2, 7]],
         [[dst_grp_stride, ngrp], [8, 7], [1, 7]]),
        (7, 1, 15, 7,
         [[src_grp_stride, ngrp], [32, 7]],
         [[dst_grp_stride, ngrp], [8, 7]]),
        (1, 7, 240, 56,
         [[src_grp_stride, ngrp], [2, 7]],
         [[dst_grp_stride, ngrp], [1, 7]]),
        (1, 1, 255, 63,
         [[src_grp_stride, ngrp], [1, 1]],
         [[dst_grp_stride, ngrp], [1, 1]]),
    ]


# ------------------------------------------------------------------ builder
def _build_nc():
    if "nc" in _CACHE:
        return _CACHE["nc"]
    nc = bacc.Bacc("TRN2", target_bir_lowering=False, debug=False,
                   enable_asserts=False, num_devices=NCORES)
    f32 = mybir.dt.float32
    f16 = mybir.dt.float16
    AO = mybir.AluOpType

    # per-core inputs
    x1s = nc.dram_tensor("x1s", (48, NGRP1 * 256), f32, kind="ExternalInput").ap()
    fw1s = nc.dram_tensor("fw1s", (4096, U), f16, kind="ExternalInput").ap()
    # shared inputs
    wb1 = nc.dram_tensor("wb1", (32, 9 * 64), f32, kind="ExternalInput").ap()
    wb2 = nc.dram_tensor("wb2", (128, 9 * 128), f32, kind="ExternalInput").ap()
    lws1 = nc.dram_tensor("lws1", (64, 9 * 64), f32, kind="ExternalInput").ap()
    lws2 = nc.dram_tensor("lws2", (64, 9 * 128), f32, kind="ExternalInput").ap()
    d2tm = nc.dram_tensor("d2tm", (128, 128), f32, kind="ExternalInput").ap()
    idt32 = nc.dram_tensor("idt32", (128, 128), f32, kind="ExternalInput").ap()
    bc1 = nc.dram_tensor("bc1", (128, 4), f32, kind="ExternalInput").ap()
    bc2 = nc.dram_tensor("bc2", (128, 2), f32, kind="ExternalInput").ap()
    b1c = nc.dram_tensor("b1c", (64, 1), f32, kind="ExternalInput").ap()
    b2c = nc.dram_tensor("b2c", (128, 1), f32, kind="ExternalInput").ap()
    ones32 = nc.dram_tensor("ones32", (1, 128), f32, kind="ExternalInput").ap()
    ones16 = nc.dram_tensor("ones16", (1, 128), f16, kind="ExternalInput").ap()
    fb1t = nc.dram_tensor("fb1t", (128, 8), f32, kind="ExternalInput").ap()
    fw2t = nc.dram_tensor("fw2t", (128, 80), f16, kind="ExternalInput").ap()
    fb2r = nc.dram_tensor("fb2r", (1, O2out), f16, kind="ExternalInput").ap()
    outt = nc.dram_tensor("outt", (NB, O2out), f32, kind="ExternalOutput").ap()
    DEBUG = bool(os.environ.get("BASSK_DEBUG"))
    if DEBUG:
        g1dump = nc.dram_tensor("g1dump", (128, NPAIR * 256), f32, kind="ExternalOutput").ap()
        g2dump = nc.dram_tensor("g2dump", (128, NB * 256), f16, kind="ExternalOutput").ap()

    with tile.TileContext(nc) as tc:
        with tc.tile_pool(name="dram", bufs=1, space="DRAM") as dram:
            g2d = dram.tile([128, NB * T], f16)          # conv2 out (o2, b*t)
            g1d = dram.tile([128, NPAIR * T], mybir.dt.float32)  # conv1 out f32
            a2out = dram.tile([128, NB * T], f16)        # alltoall result
            rs_in = dram.tile([B, U], f32)
            rs_out = dram.tile([NB, U], f32)

            # ======================================================== conv
            with tc.tile_pool(name="consts", bufs=1) as cst, \
                 tc.tile_pool(name="xp", bufs=1) as xp, \
                 tc.tile_pool(name="g1p", bufs=1) as g1p, \
                 tc.tile_pool(name="g2p", bufs=1) as g2p, \
                 tc.tile_pool(name="wk", bufs=2) as wk, \
                 tc.tile_pool(name="tcmp", bufs=8) as tcmp, \
                 tc.tile_pool(name="psc", bufs=2, space="PSUM") as psc_p, \
                 tc.tile_pool(name="ptp", bufs=2, space="PSUM") as ptp_p, \
                 tc.tile_pool(name="pws", bufs=1, space="PSUM") as pws_p, \
                 tc.tile_pool(name="pagg", bufs=2, space="PSUM") as pagg_p:

                def ctile(nm, shape, dt_, src):
                    t_ = cst.tile(shape, dt_, name=nm, tag=nm)
                    nc.sync.dma_start(t_[:], src[:, :])
                    return t_

                idt32t = ctile("idt32t", [128, 128], f32, idt32)
                d2tmt = ctile("d2tmt", [128, 128], f32, d2tm)
                wb1t = ctile("wb1t", [32, 576], f32, wb1)
                wb2t = ctile("wb2t", [128, 1152], f32, wb2)
                lws1t = ctile("lws1t", [64, 576], f32, lws1)
                lws2t = ctile("lws2t", [64, 1152], f32, lws2)
                bc1t = ctile("bc1t", [128, 4], f32, bc1)
                bc2t = ctile("bc2t", [128, 2], f32, bc2)
                b1ct = ctile("b1ct", [64, 1], f32, b1c)
                b2ct = ctile("b2ct", [128, 1], f32, b2c)
                ones32t = ctile("ones32t", [1, 128], f32, ones32)
                ones16t = ctile("ones16t", [1, 128], f16, ones16)


                # ---------------- generic conv layer ----------------
                def conv_layer(layer):
                    if layer == 1:
                        nblk, bstr, ngrp, O = 4, 32, NGRP1, 64   # blockdiag count, block stride
                        xdram, wbt, lwst, bct, bcol = x1s, wb1t, lws1t, b1ct, bc1t
                    else:
                        nblk, bstr, ngrp, O = 2, 64, NPAIR, 128
                        xdram, wbt, lwst, bct, bcol = g1d, wb2t, lws2t, b2ct, bc2t
                    P = 128
                    NPG = nblk // 2                               # pairs per group
                    WSW = 9 * O                                   # ws width

                    for grp in range(ngrp):
                        gw = nblk * 64
                        # stream this group's tokens from DRAM
                        xg = wk.tile([128, 256], f32, tag="xg", bufs=3)
                        if layer == 1:
                            nc.vector.memset(xg[:], 0.0)
                            for g in range(4):
                                nc.sync.dma_start(
                                    xg[32 * g:32 * g + 12, :],
                                    xdram[12 * g:12 * (g + 1), grp * 256:(grp + 1) * 256])
                        else:
                            nc.sync.dma_start(xg[:], xdram[:, grp * 256:(grp + 1) * 256])
                        # candidate gather for this group (unscaled f32)
                        ss = wk.tile([128, 64], f32, tag="sscur", bufs=3)
                        for (na, nbp, so, do, sp, dp) in _cand_copy_pieces(1, 256, 64):
                            nc.vector.tensor_copy(
                                fap(ss[0:P, do:do + 1], dp[1:]),
                                fap(xg[0:P, so:so + 1], sp[1:]))
                        # block-diagonal lhs (2x scale folded here)
                        bd = wk.tile([P, gw], f32, tag="bd")
                        nc.vector.memset(bd[:], 0.0)
                        for g in range(nblk):
                            nc.vector.tensor_scalar(
                                bd[bstr * g:bstr * (g + 1), 64 * g:64 * (g + 1)],
                                ss[bstr * g:bstr * (g + 1), 0:64],
                                2.0, None, op0=AO.mult)
                        # squares + -s2 row (1, gw)
                        sq = wk.tile([P, 64], f32, tag="sq")
                        nc.vector.tensor_tensor(sq[:], ss[:, 0:64], ss[:, 0:64], op=AO.mult)
                        ps2t = ptp_p.tile([nblk, 64], f32, tag="ptp")
                        nc.tensor.matmul(ps2t[:], lhsT=bcol[:], rhs=sq[:],
                                         start=True, stop=True)
                        s2sb = wk.tile([nblk, 64], f32, tag="s2sb")
                        nc.scalar.copy(s2sb[:], ps2t[:])
                        s2fl = wk.tile([1, gw], f32, tag="s2fl")
                        nc.sync.dma_start(
                            fap(s2fl[0:1, 0:1], [[64, nblk], [1, 64]]), s2sb[:])

                        tcms = [tcmp.tile([128, 256], f32, tag="tcm", name=f"tcm{layer}_{grp}_{i}")
                                for i in range(NPG)]
                        for half in range(2):
                            psc = psc_p.tile([128, gw], f32, tag="psc")
                            nc.tensor.matmul(
                                psc[:], lhsT=xg[:, half * 128:half * 128 + 128],
                                rhs=bd[:], start=True, stop=False)
                            nc.tensor.matmul(psc[:], lhsT=ones32t[:, 0:128],
                                             rhs=s2fl[:], start=False, stop=True)
                            negsc = wk.tile([128, gw], f32, tag="negsc")
                            nc.vector.tensor_tensor(
                                fap(negsc[0:128, 0:1], [[64, nblk], [1, 64]]),
                                fap(psc[0:128, 0:1], [[64, nblk], [1, 64]]),
                                fap(d2tmt[0:128, half * 64:half * 64 + 1], [[0, nblk], [1, 64]]),
                                op=AO.subtract)
                            vbuf = wk.tile([128, nblk * 16], f32, tag="vbuf")
                            mrt = wk.tile([128, gw], f32, tag="mrt")
                            for g in range(nblk):
                                nc.vector.max(out=vbuf[:, g * 16:g * 16 + 8],
                                              in_=negsc[:, g * 64:(g + 1) * 64])
                                nc.vector.match_replace(
                                    out=mrt[:, g * 64:(g + 1) * 64],
                                    in_to_replace=vbuf[:, g * 16:g * 16 + 8],
                                    in_values=negsc[:, g * 64:(g + 1) * 64],
                                    imm_value=-1e30)
                                nc.vector.max(out=vbuf[:, g * 16 + 8:g * 16 + 16],
                                              in_=mrt[:, g * 64:(g + 1) * 64])
                            for pq in range(NPG):
                                Ct = wk.tile([128, 1152], f16, tag="ct")
                                nc.vector.tensor_tensor(
                                    fap(Ct[0:128, 0:1], [[576, 2], [9, 64], [1, 9]]),
                                    fap(negsc[0:128, pq * 128:pq * 128 + 1], [[64, 2], [1, 64], [0, 9]]),
                                    fap(vbuf[0:128, pq * 32:pq * 32 + 1], [[16, 2], [0, 64], [1, 9]]),
                                    op=AO.is_lt)
                                Tt = wk.tile([128, 128], f32, tag="tt")
                                nc.vector.tensor_reduce(
                                    Tt[:], fap(Ct[0:128, 0:1], [[9, 128], [1, 9]]),
                                    axis=mybir.AxisListType.X, op=AO.add)
                                ptp = ptp_p.tile([128, 128], f32, tag="ptp")
                                nc.tensor.transpose(ptp[:], Tt[:], idt32t[:])
                                nc.scalar.copy(tcms[pq][:, half * 128:half * 128 + 128], ptp[:])

                        for pq in range(NPG):
                            pairg = grp * NPG + pq
                            Tcm = tcms[pq]
                            M = wk.tile([128, 9 * 256], f32, tag="m")
                            selt = wk.tile([128, 256], f32, tag="selt")
                            tca = wk.tile([128, 256], f32, tag="tca")
                            tcb = wk.tile([128, 256], f32, tag="tcb")
                            nc.vector.tensor_scalar(M[:, 0:256], Tcm[:], 4.5, None, op0=AO.is_le)
                            nc.vector.tensor_scalar(selt[:], Tcm[:], 8.5, None, op0=AO.is_le)
                            nc.vector.tensor_tensor(M[:, 5 * 256:6 * 256], selt[:], M[:, 0:256], op=AO.subtract)
                            nc.vector.tensor_scalar(tca[:], Tcm[:], -2.0, None, op0=AO.add)
                            nc.vector.tensor_scalar(tcb[:], Tcm[:], -6.0, None, op0=AO.add)
                            for p in range(1, 5):
                                nc.vector.tensor_tensor(M[:, p * 256:(p + 1) * 256],
                                                        M[:, (p - 1) * 256:p * 256], tca[:], op=AO.mult)
                            for p in range(6, 9):
                                nc.vector.tensor_tensor(M[:, p * 256:(p + 1) * 256],
                                                        M[:, (p - 1) * 256:p * 256], tcb[:], op=AO.mult)
                            ws = wk.tile([128, WSW], f32, tag="ws")
                            for gg in range(2):
                                g = pq * 2 + gg
                                b = grp * nblk + g
                                if layer == 1:
                                    sswk = wk.tile([32, 64], f32, tag="sswk")
                                    nc.vector.tensor_copy(
                                        sswk[:], ss[bstr * g:bstr * (g + 1), 0:64])
                                    lhs_ws = sswk[:]
                                    rhs_ws = wbt
                                else:
                                    lhs_ws = ss[bstr * g:bstr * (g + 1), 0:64]
                                    rhs_ws = wbt[bstr * g:bstr * (g + 1), :]
                                for piece in range((WSW + 511) // 512):
                                    lo = piece * 512
                                    hi = min(lo + 512, WSW)
                                    pws = pws_p.tile([64, 512], f32, tag="pws")
                                    nc.tensor.matmul(pws[:, 0:hi - lo], lhsT=lhs_ws,
                                                     rhs=rhs_ws[:, lo:hi],
                                                     start=True, stop=True)
                                    nc.vector.tensor_tensor(
                                        ws[64 * gg:64 * gg + 64, lo:hi],
                                        pws[:, 0:hi - lo], lwst[:, lo:hi], op=AO.add)
                                pagg = pagg_p.tile([O, 256], f32, tag="pagg")
                                for p in range(9):
                                    nc.tensor.matmul(pagg[:], lhsT=ws[64 * gg:64 * gg + 64,
                                                                      p * O:(p + 1) * O],
                                                     rhs=M[64 * gg:64 * gg + 64, p * 256:(p + 1) * 256],
                                                     start=(p == 0), stop=(p == 8))
                                if layer == 1:
                                    if gg == 0:
                                        g1stg = wk.tile([128, 256], mybir.dt.float32,
                                                        tag="g1stg", name=f"g1stg_{grp}_{pq}")
                                    nc.scalar.activation(
                                        g1stg[64 * gg:64 * gg + 64, :], pagg[:],
                                        mybir.ActivationFunctionType.Relu, bias=bct[:, 0:1])
                                    if gg == 1:
                                        nc.sync.dma_start(
                                            g1d[:, pairg * 256:(pairg + 1) * 256], g1stg[:])
                                else:
                                    g2stg = wk.tile([128, 256], mybir.dt.float16, tag="g2stg")
                                    nc.scalar.activation(g2stg[:], pagg[:],
                                                         mybir.ActivationFunctionType.Relu,
                                                         bias=bct[:, 0:1])
                                    nc.sync.dma_start(g2d[:, b * 256:(b + 1) * 256], g2stg[:])

                conv_layer(1)
                conv_layer(2)
                if DEBUG:
                    nc.sync.dma_start(g1dump[:, :], g1d[:, :])
                    nc.sync.dma_start(g2dump[:, :], g2d[:, :])

            nc.gpsimd.collective_compute(
                "AllToAll", mybir.AluOpType.bypass,
                replica_groups=[list(range(NCORES))],
                ins=[g2d.opt()], outs=[a2out.opt()],
            )

            # ======================================================== fc
            f32 = mybir.dt.float32
            f16 = mybir.dt.float16
            with tc.tile_pool(name="fcw", bufs=1) as fcw, \
                 tc.tile_pool(name="fcs", bufs=2) as fcs, \
                 tc.tile_pool(name="cst2", bufs=1) as cst2, \
                 tc.tile_pool(name="pfc", bufs=2, space="PSUM") as pfc_p, \
                 tc.tile_pool(name="ptp2", bufs=2, space="PSUM") as ptp2_p:

                idt32b = cst2.tile([128, 128], f32)
                nc.sync.dma_start(idt32b[:], idt32[:, :])
                fb1tb = cst2.tile([128, 8], f32)
                nc.sync.dma_start(fb1tb[:], fb1t[:, :])
                fw2tb = cst2.tile([128, 80], f16)
                nc.sync.dma_start(fw2tb[:], fw2t[:, :])
                fb2rb = cst2.tile([1, O2out], f16)
                nc.sync.dma_start(fb2rb[:], fb2r[:, :])
                ones16b = cst2.tile([1, 128], f16)
                nc.sync.dma_start(ones16b[:], ones16[:, :])

                fw1sb = fcw.tile([128, 32 * U], f16)
                nc.sync.dma_start(
                    fw1sb[:],
                    fap(fw1s[0:128, 0:1], [[U * 128, 32], [1, U]]))
                h2sb = fcw.tile([128, 32 * U], f16)
                # restack alltoall output: chunk c = (o2r=c//2, t-half c%2);
                # a2out[16j+o2r, b*256+t]; chunk partitions = t-half, free (j, b)
                for c in range(32):
                    for j in range(8):
                        sl = AP(tensor=a2out.tensor,
                                offset=a2out[16 * j + c // 2:16 * j + c // 2 + 1,
                                             (c % 2) * 128:(c % 2) * 128 + 1].offset,
                                ap=[[1, 128], [256, 128]])
                        nc.sync.dma_start(h2sb[:, c * U + j * 128:c * U + (j + 1) * 128], sl)

                for bt in range(8):
                    for uh in range(2):
                        pfc = pfc_p.tile([128, 512], f32, tag="pfc")
                        for c in range(32):
                            nc.tensor.matmul(
                                pfc[:], lhsT=h2sb[:, c * U + bt * 128:c * U + bt * 128 + 128],
                                rhs=fw1sb[:, c * U + uh * 512:c * U + uh * 512 + 512],
                                start=(c == 0), stop=(c == 31))
                        stg = fcs.tile([128, 512], f32, tag="stg")
                        nc.scalar.copy(stg[:], pfc[:])
                        nc.sync.dma_start(
                            rs_in[bt * 128:(bt + 1) * 128, uh * 512:(uh + 1) * 512], stg[:])

                nc.gpsimd.collective_compute(
                    "ReduceScatter", mybir.AluOpType.add,
                    replica_groups=[list(range(NCORES))],
                    ins=[rs_in.opt()], outs=[rs_out.opt()],
                )

                h1raw = fcs.tile([128, U], f32, tag="h1raw")
                nc.sync.dma_start(h1raw[:], rs_out[:, :])
                h1T = fcs.tile([128, U], f16, tag="h1T")
                for c in range(8):
                    ptp2 = ptp2_p.tile([128, 128], f32, tag="ptp2")
                    nc.tensor.transpose(ptp2[:], h1raw[:, c * 128:(c + 1) * 128], idt32b[:])
                    nc.scalar.activation(h1T[:, c * 128:(c + 1) * 128], ptp2[:],
                                         mybir.ActivationFunctionType.Relu,
                                         bias=fb1tb[:, c:c + 1])
                psum2 = ptp2_p.tile([128, O2out], f32, tag="psum2b")
                for c in range(8):
                    nc.tensor.matmul(psum2[:], lhsT=h1T[:, c * 128:(c + 1) * 128],
                                     rhs=fw2tb[:, c * O2out:(c + 1) * O2out],
                                     start=(c == 0), stop=False)
                nc.tensor.matmul(psum2[:], lhsT=ones16b[:], rhs=fb2rb[:],
                                 start=False, stop=True)
                out_t = fcs.tile([128, O2out], f32, tag="outf")
                nc.scalar.copy(out_t[:], psum2[:])
                nc.sync.dma_start(outt[:, :], out_t[:])

    nc.compile()
    _CACHE["nc"] = nc
    return nc


# ------------------------------------------------------------------ host prep
def _host_shared(w1, b1, w2, b2, fb1, fw2, fb2):
    pos = np.linspace(0., 1., 16).astype(np.float32)
    tt = np.arange(T)
    ly, lx = pos[tt // 16], pos[tt % 16]
    IH = np.linspace(0, 15, 8).astype(np.int32)
    cand_t = (IH[:, None] * 16 + IH[None, :]).reshape(-1)
    cy, cx = ly[cand_t], lx[cand_t]
    d2loc = (ly[:, None] - cy[None, :]) ** 2 + (lx[:, None] - cx[None, :]) ** 2
    d2tm = np.empty((128, 128), np.float32)
    for half in range(2):
        d2tm[:, half * 64:(half + 1) * 64] = d2loc[half * 128:(half + 1) * 128, :]

    VA = np.array([[(r - 2) ** p for p in range(5)] for r in range(5)], np.float64)
    CA = np.linalg.inv(VA)
    VB = np.array([[(r - 6) ** p for p in range(4)] for r in range(5, 9)], np.float64)
    CB = np.linalg.inv(VB)

    def basis(w):  # w (O, Cf, 9) -> Wb (9, O, Cf)
        O, Cf, _ = w.shape
        Wb = np.zeros((9, O, Cf), np.float64)
        for k in range(9):
            if k <= 4:
                for p in range(5):
                    Wb[p] += CA[p, k] * w[:, :, k]
            else:
                for p in range(4):
                    Wb[5 + p] += CB[p, k - 5] * w[:, :, k]
        return Wb

    Wb1 = basis(np.asarray(w1, np.float64))     # (9, 64, 14)
    Wb2 = basis(np.asarray(w2, np.float64))     # (9, 128, 66)

    # feature part, halved (samples are 2x-scaled), replicated per block
    wb1r = np.zeros((32, 576), np.float32)
    for p in range(9):
        wb1r[:12, p * 64:(p + 1) * 64] = Wb1[p, :, :12].T
    wb2r = np.zeros((128, 1152), np.float32)
    for g in range(2):
        for p in range(9):
            wb2r[g * 64:(g + 1) * 64, p * 128:(p + 1) * 128] = \
                Wb2[p, :, :64].T
    # location part: lws[n, p*O+o] = sum_l locval[l,n] * Wb[p,o,Cfeat+l]
    locv = np.stack([cy, cx])                    # (2, 64)
    lws1 = np.zeros((64, 576), np.float32)
    lws2 = np.zeros((64, 1152), np.float32)
    for p in range(9):
        lws1[:, p * 64:(p + 1) * 64] = locv.T @ Wb1[p, :, 12:].T
        lws2[:, p * 128:(p + 1) * 128] = locv.T @ Wb2[p, :, 64:].T

    bc1 = np.zeros((128, 4), np.float32)
    for g in range(4):
        bc1[g * 32:g * 32 + 12, g] = -1.0
    bc2 = np.zeros((128, 2), np.float32)
    for g in range(2):
        bc2[g * 64:(g + 1) * 64, g] = -1.0

    fw2 = np.asarray(fw2, np.float32)
    fw2t = fw2.T.reshape(8, 128, O2out).transpose(1, 0, 2).reshape(128, 80)
    return dict(
        wb1=wb1r, wb2=wb2r,
        lws1=lws1, lws2=lws2, d2tm=d2tm,
        idt32=np.eye(128, dtype=np.float32),
        bc1=bc1, bc2=bc2,
        b1c=np.asarray(b1, np.float32).reshape(64, 1),
        b2c=np.asarray(b2, np.float32).reshape(128, 1),
        ones32=np.ones((1, 128), np.float32),
        ones16=np.ones((1, 128), F16),
        fb1t=np.ascontiguousarray(np.asarray(fb1, np.float32).reshape(8, 128).T),
        fw2t=fw2t.astype(F16),
        fb2r=np.asarray(fb2, np.float32).reshape(1, O2out).astype(F16),
    )


def _phi():
    O2v, HH, WW = np.meshgrid(np.arange(128), np.arange(16), np.arange(16),
                              indexing="ij")
    C2 = O2v // 4
    I = (O2v % 4) // 2
    J = O2v % 2
    return (C2 * 1024 + (2 * HH + I) * 32 + (2 * WW + J)).reshape(-1)


def kernel(x, w1, b1, w2, b2, fw1, fb1, fw2, fb2):
    import time as _time
    import sys as _sys
    _t0 = _time.time()

    def _mark(label):
        print(f"[kernel] {label}: {_time.time() - _t0:.2f}s", file=_sys.stderr, flush=True)

    nc = _build_nc()
    _mark("bass ready")

    x = np.asarray(x, np.float32)
    xu = x.reshape(B, 3, 16, 2, 16, 2).transpose(0, 1, 3, 5, 2, 4).reshape(B, 12, 256)
    shared = _host_shared(w1, b1, w2, b2, fb1, fw2, fb2)
    phi = _phi()
    fw1p = np.asarray(fw1, np.float32).T[phi].astype(F16)   # (32768, 1024)
    _mark("host prep")

    in_maps = []
    for i in range(NCORES):
        xc = xu[i * NB:(i + 1) * NB]
        x1stack = np.ascontiguousarray(
            xc.reshape(NGRP1, 4, 12, 256).transpose(1, 2, 0, 3).reshape(48, NGRP1 * 256))
        m = dict(shared)
        m["x1s"] = x1stack
        m["fw1s"] = fw1p[i * 4096:(i + 1) * 4096]
        in_maps.append(m)
    _mark("in_maps")

    res = run_bass_kernel_spmd(nc, in_maps, core_ids=list(range(NCORES)))
    global _LAST_RES
    _LAST_RES = res
    _mark("spmd run")
    out = np.empty((B, O2out), np.float32)
    for i in range(NCORES):
        out[i * NB:(i + 1) * NB] = res.results[i]["outt"]
    return out
